# revision 25
# baseline (speedup 1.0000x reference)
"""2-layer GAT (PyG GATConv-style) on 8 Trainium2 NeuronCores.

Sharding: nodes padded to 160 tiles of 128; core c owns dst-tiles
[20c, 20c+20). Edges (incl. self-loops) live on the core owning their
destination, sorted by dst, padded to 128-edge chunks (G[j] chunks for
dst-tile slot j, identical across cores so all cores run one program).

Layer 1 needs h[src] = (x @ W1)[src] per edge; instead of gathering on
device, the host uploads x.T[:, src(e)] per edge (xeT) and the kernel
computes per-edge hidden rows with a per-chunk GEMM in bf16 — no
dma_gather in layer 1 (gpsimd dma_gather costs ~8ns/idx, measured).
Segment-softmax and scatter-aggregate run per dst-tile as one-hot bf16
matmuls (s01 scatter / t01 expand, host-built, streamed as bf16).

Between layers, per-node rows z = [h2@W2 | asrc2 | adst2] (bf16, padded
to 256B) are AllGathered in 4 groups of 5 dst-tiles so the collective
overlaps layer-1 compute. Layer 2 fetches z[src(e)] per edge with
dma_gather (unavoidable; index-rate-bound) and aggregates with the same
one-hot matmuls.
"""

import os

import numpy as np
import ml_dtypes

import concourse.bass as bass
import concourse.mybir as mybir
import concourse.tile as tile
from concourse import bacc
from concourse.bass_utils import run_bass_kernel_spmd

dt = mybir.dt
bf16 = ml_dtypes.bfloat16

N_CORES = 8
N_NODES = 20000
IN_CH = 128
HID = 32
HEADS = 8
HC = HEADS * HID  # 256
OUT_CH = 64
NEG_SLOPE = 0.2

P = 128
TILES_PER_CORE = 20
N_PAD = N_CORES * TILES_PER_CORE * P  # 20480
LOC_NODES = TILES_PER_CORE * P  # 2560
NGRP = 4                  # AllGather groups
TPG = TILES_PER_CORE // NGRP  # tiles per group
ROWZ = 128                # z row: [h2W2(64) | s2 | t2 | pad] bf16 -> 256B

AF = mybir.ActivationFunctionType
OP = mybir.AluOpType


def _prep_edges(edge_index):
    src = np.asarray(edge_index[0], dtype=np.int64)
    dst = np.asarray(edge_index[1], dtype=np.int64)
    loops = np.arange(N_NODES, dtype=np.int64)
    src = np.concatenate([src, loops])
    dst = np.concatenate([dst, loops])

    order = np.lexsort((src, dst))
    src, dst = src[order], dst[order]
    tile_of = dst // P
    core_of = np.minimum(tile_of // TILES_PER_CORE, N_CORES - 1)

    per = [[None] * TILES_PER_CORE for _ in range(N_CORES)]
    for c in range(N_CORES):
        mc = core_of == c
        sc, tc_, dc = src[mc], tile_of[mc], dst[mc]
        for j in range(TILES_PER_CORE):
            gt = c * TILES_PER_CORE + j
            mt = tc_ == gt
            per[c][j] = (sc[mt], dc[mt] - gt * P)

    G = [max(max(1, -(-len(per[c][j][0]) // P)) for c in range(N_CORES))
         for j in range(TILES_PER_CORE)]

    # layer-2: edges of (core, dst-tile) split by src AllGather group
    per2 = [[[None] * NGRP for _ in range(TILES_PER_CORE)]
            for _ in range(N_CORES)]
    for c in range(N_CORES):
        for j in range(TILES_PER_CORE):
            s, dl = per[c][j]
            qof = (s % LOC_NODES) // (TPG * P)
            for q in range(NGRP):
                m = qof == q
                per2[c][j][q] = (s[m], dl[m])
    G2 = [[max(max(1, -(-len(per2[c][j][q][0]) // P)) for c in range(N_CORES))
           for q in range(NGRP)] for j in range(TILES_PER_CORE)]

    def onehots(dlp):
        s01 = (dlp[:, None] == np.arange(P)[None, :])
        return s01.astype(bf16), np.ascontiguousarray(s01.T).astype(bf16)

    meta = []
    for c in range(N_CORES):
        srcs, s01s, t01s = [], [], []
        for j in range(TILES_PER_CORE):
            s, dl = per[c][j]
            n_pad = G[j] * P
            sp = np.zeros(n_pad, dtype=np.int64)
            sp[: len(s)] = s
            srcs.append(sp)
            dlp = np.full(n_pad, 200, dtype=np.int64)
            dlp[: len(dl)] = dl
            for g in range(G[j]):
                a, b = onehots(dlp[g * P:(g + 1) * P])
                s01s.append(a)
                t01s.append(b)
        idx2_cols, s01s2, t01s2 = [], [], []
        for q in range(NGRP):
            for j in range(TILES_PER_CORE):
                s, dl = per2[c][j][q]
                n_pad = G2[j][q] * P
                sp = np.zeros(n_pad, dtype=np.int64)
                sp[: len(s)] = s
                dlp = np.full(n_pad, 200, dtype=np.int64)
                dlp[: len(dl)] = dl
                # row within h2ag[q]: rank-major
                rows = (sp // LOC_NODES) * (TPG * P) + (sp % LOC_NODES) - q * TPG * P
                rows[len(s):] = 0
                idx16 = rows.astype(np.int16).reshape(n_pad // 16, 16).T
                idx2_cols.append(np.tile(idx16, (8, 1)))
                for g in range(G2[j][q]):
                    a, b = onehots(dlp[g * P:(g + 1) * P])
                    s01s2.append(a)
                    t01s2.append(b)
        meta.append({
            "src": np.concatenate(srcs),
            "s01": np.ascontiguousarray(np.stack(s01s, 1).reshape(P, -1)),
            "t01": np.ascontiguousarray(np.stack(t01s, 1).reshape(P, -1)),
            "idx2": np.ascontiguousarray(np.concatenate(idx2_cols, axis=1)),
            "s012": np.ascontiguousarray(np.stack(s01s2, 1).reshape(P, -1)),
            "t012": np.ascontiguousarray(np.stack(t01s2, 1).reshape(P, -1)),
        })
    return G, G2, meta


def _build_program(G, G2):
    NCH = sum(G)
    NCH2 = sum(G2[j][q] for j in range(TILES_PER_CORE) for q in range(NGRP))
    nc = bacc.Bacc(None, target_bir_lowering=False, debug=False)

    xeT = nc.dram_tensor("xeT", [P, NCH * P], dt.bfloat16, kind="ExternalInput")
    xlT = nc.dram_tensor("xlT", [P, LOC_NODES], dt.bfloat16, kind="ExternalInput")
    W1a = nc.dram_tensor("W1a", [P, HC + 16], dt.bfloat16, kind="ExternalInput")
    W2a = nc.dram_tensor("W2a", [HC, OUT_CH + 2], dt.bfloat16, kind="ExternalInput")
    b1r = nc.dram_tensor("b1r", [P, HC], dt.float32, kind="ExternalInput")
    b2r = nc.dram_tensor("b2r", [P, OUT_CH], dt.float32, kind="ExternalInput")
    s01T = nc.dram_tensor("s01", [P, NCH * P], dt.bfloat16, kind="ExternalInput")
    t01T = nc.dram_tensor("t01", [P, NCH * P], dt.bfloat16, kind="ExternalInput")
    idx2T = nc.dram_tensor("idx2", [P, 8 * NCH2], dt.int16, kind="ExternalInput")
    s012T = nc.dram_tensor("s012", [P, NCH2 * P], dt.bfloat16, kind="ExternalInput")
    t012T = nc.dram_tensor("t012", [P, NCH2 * P], dt.bfloat16, kind="ExternalInput")
    outT = nc.dram_tensor("out", [LOC_NODES, OUT_CH], dt.float32, kind="ExternalOutput")

    coff = [0]
    for j in range(1, TILES_PER_CORE + 1):
        coff.append(coff[-1] + G[j - 1])
    coff2 = {}
    off = 0
    for q in range(NGRP):
        for j in range(TILES_PER_CORE):
            coff2[(q, j)] = off
            off += G2[j][q]
    maxG2 = max(G2[j][q] for j in range(TILES_PER_CORE) for q in range(NGRP))

    with tile.TileContext(nc) as tc:
        with (
            tc.tile_pool(name="const", bufs=1) as cp,
            tc.tile_pool(name="stage", bufs=3) as stp,
            tc.tile_pool(name="edges", bufs=3) as eps,
            tc.tile_pool(name="edges3", bufs=4) as ep3,
            tc.tile_pool(name="ps", bufs=1, space="PSUM") as pp,
            tc.tile_pool(name="dram", bufs=1, space="DRAM") as dp,
        ):
            # ---- constants ----
            w1a_sb = cp.tile([P, HC + 16], dt.bfloat16)
            nc.sync.dma_start(w1a_sb[:], W1a[:, :])
            w2a_sb = cp.tile([P, 2, OUT_CH + 2], dt.bfloat16)
            nc.sync.dma_start(w2a_sb[:],
                              W2a[:, :].rearrange("(k p) n -> p k n", p=P))
            b1_sb = cp.tile([P, HC], dt.float32)
            nc.sync.dma_start(b1_sb[:], b1r[:, :])
            b2_sb = cp.tile([P, OUT_CH], dt.float32)
            nc.sync.dma_start(b2_sb[:], b2r[:, :])
            idx2_sb = cp.tile([P, 8 * NCH2], dt.int16)
            nc.sync.dma_start(idx2_sb[:], idx2T[:, :])
            xl_sb = cp.tile([P, LOC_NODES], dt.bfloat16)
            nc.sync.dma_start(xl_sb[:], xlT[:, :])

            iot_row = cp.tile([P, P], dt.float32)
            nc.gpsimd.iota(iot_row[:], pattern=[[1, P]], base=0,
                           channel_multiplier=0,
                           allow_small_or_imprecise_dtypes=True)
            iot_col = cp.tile([P, 1], dt.float32)
            nc.gpsimd.iota(iot_col[:], pattern=[[0, 1]], base=0,
                           channel_multiplier=1,
                           allow_small_or_imprecise_dtypes=True)
            ident = cp.tile([P, P], dt.bfloat16)
            nc.vector.tensor_scalar(ident[:], iot_row[:], iot_col[:], None,
                                    op0=OP.is_equal)

            h2rows = dp.tile([LOC_NODES, ROWZ], dt.bfloat16)
            h2ag = [dp.tile([N_CORES * TPG * P, ROWZ], dt.bfloat16,
                            addr_space="Shared", name=f"h2ag{q}")
                    for q in range(NGRP)]

            # ---- local adst1 per dst-tile (from local x) ----
            loc_adst = cp.tile([P, TILES_PER_CORE, 8], dt.bfloat16)
            for j in range(TILES_PER_CORE):
                psL = pp.tile([P, 8], dt.float32, tag="ps2", bufs=1,
                              padded_shape=[P, OUT_CH + 2])
                nc.tensor.matmul(psL[:], xl_sb[:, j * P:(j + 1) * P],
                                 w1a_sb[:, HC + 8:HC + 16], start=True, stop=True)
                nc.vector.tensor_copy(loc_adst[:, j, :], psL[:])
            loc_adst2 = cp.tile([P, TILES_PER_CORE, 1], dt.bfloat16)

            # ---- layer 2 sub-tile emission (interleaved with layer 1) ----
            acc2 = cp.tile([P, TILES_PER_CORE, OUT_CH + 1], dt.float32)

            maxB = max(sum(G2[jj][q] for jj in range(b * TPG,
                                                      min((b + 1) * TPG,
                                                          TILES_PER_CORE)))
                       for q in range(NGRP)
                       for b in range(-(-TILES_PER_CORE // TPG)))
            blocks = {}

            def ensure_block(q, j):
                b = j // TPG
                if (q, b) in blocks:
                    return
                jlo = b * TPG
                jhi = min((b + 1) * TPG, TILES_PER_CORE)
                nchb = sum(G2[jj][q] for jj in range(jlo, jhi))
                off0 = coff2[(q, jlo)]
                nib = nchb * P
                hgb = ep3.tile([P, nchb, ROWZ], dt.bfloat16, tag="hg2",
                               padded_shape=[P, maxB, ROWZ])
                nc.gpsimd.dma_gather(hgb[:, 0:nchb, :], h2ag[q][:, :],
                                     idx2_sb[:, 8 * off0:8 * (off0 + nchb)],
                                     num_idxs=nib, num_idxs_reg=nib,
                                     elem_size=ROWZ, single_packet=False)
                blocks[(q, b)] = (hgb, off0)

            def emit_l2(q, j):
                Gq = G2[j][q]
                off = coff2[(q, j)]
                ensure_block(q, j)
                hgb, off0 = blocks[(q, j // TPG)]
                lo = off - off0
                hg2 = hgb[:, lo:lo + Gq, :]
                s01 = ep3.tile([P, Gq, P], dt.bfloat16, tag="s01b",
                               padded_shape=[P, maxG2, P])
                nc.sync.dma_start(s01[:, 0:Gq, :],
                                  s012T[:, off * P:(off + Gq) * P]
                                  .rearrange("p (g e) -> p g e", e=P))
                t01 = ep3.tile([P, Gq, P], dt.bfloat16, tag="t01b",
                               padded_shape=[P, maxG2, P])
                nc.sync.dma_start(t01[:, 0:Gq, :],
                                  t012T[:, off * P:(off + Gq) * P]
                                  .rearrange("p (g e) -> p g e", e=P))

                mg2 = eps.tile([P, Gq, OUT_CH + 1], dt.bfloat16, tag="mg2",
                               padded_shape=[P, maxG2, OUT_CH + 1])
                a2t = stp.tile([P, Gq, 1], dt.float32, tag="a2t",
                               padded_shape=[P, maxG2, 1])
                for g in range(Gq):
                    ps_a2 = pp.tile([P, 1], dt.float32, tag="psG", bufs=3,
                                    padded_shape=[P, HC + 8])
                    nc.tensor.matmul(ps_a2[:], t01[:, g, :],
                                     loc_adst2[:, j, :], start=True, stop=True)
                    nc.vector.tensor_tensor(a2t[:, g, :],
                                            hg2[:, g, OUT_CH:OUT_CH + 1],
                                            ps_a2[:], op=OP.add)
                e2b = stp.tile([P, Gq], dt.float32, tag="e2b",
                               padded_shape=[P, maxG2])
                nc.vector.scalar_tensor_tensor(e2b[:], a2t[:, :, 0], NEG_SLOPE,
                                               a2t[:, :, 0],
                                               op0=OP.mult, op1=OP.max)
                al2t = stp.tile([P, Gq], dt.float32, tag="al2t",
                                padded_shape=[P, maxG2])
                nc.scalar.activation(al2t[:], e2b[:], AF.Exp)
                nc.vector.tensor_tensor(
                    mg2[:, :, 0:OUT_CH], hg2[:, 0:Gq, 0:OUT_CH],
                    al2t[:].unsqueeze(2).broadcast_to([P, Gq, OUT_CH]),
                    op=OP.mult)
                nc.vector.tensor_copy(mg2[:, :, OUT_CH:OUT_CH + 1],
                                      al2t[:].unsqueeze(2))
                ps3 = pp.tile([P, OUT_CH + 1], dt.float32, tag="ps_o", bufs=3,
                              padded_shape=[P, HC + 8])
                for g in range(Gq):
                    nc.tensor.matmul(ps3[:], s01[:, g, :], mg2[:, g, :],
                                     start=(g == 0), stop=(g == Gq - 1))
                if q == 0:
                    nc.vector.tensor_copy(acc2[:, j, :], ps3[:])
                else:
                    nc.vector.tensor_tensor(acc2[:, j, :], acc2[:, j, :],
                                            ps3[:], op=OP.add)
                if q == NGRP - 1:
                    den2 = stp.tile([P, 1], dt.float32, tag="den2")
                    nc.vector.tensor_scalar(den2[:],
                                            acc2[:, j, OUT_CH:OUT_CH + 1],
                                            1e-16, None, op0=OP.add)
                    rec2 = stp.tile([P, 1], dt.float32, tag="rec2")
                    nc.vector.reciprocal(rec2[:], den2[:])
                    o2 = stp.tile([P, OUT_CH], dt.float32, tag="o2")
                    nc.vector.tensor_tensor(o2[:], acc2[:, j, 0:OUT_CH],
                                            rec2[:].broadcast_to([P, OUT_CH]),
                                            op=OP.mult)
                    o2b = stp.tile([P, OUT_CH], dt.float32, tag="o2b")
                    nc.vector.tensor_tensor(o2b[:], o2[:], b2_sb[:], op=OP.add)
                    nc.sync.dma_start(
                        outT[:, :].rearrange("(t p) r -> p t r", p=P)[:, j, :],
                        o2b[:])

            pending = [(q2, j2) for q2 in range(NGRP)
                       for j2 in range(TILES_PER_CORE)]

            # ---- layer 1 ----
            for j in range(TILES_PER_CORE):
                Gj = G[j]
                xe = ep3.tile([P, Gj, P], dt.bfloat16, tag="xe",
                              padded_shape=[P, max(G), P])
                nc.sync.dma_start(xe[:, 0:Gj, :],
                                  xeT[:, coff[j] * P:(coff[j] + Gj) * P]
                                  .rearrange("p (g e) -> p g e", e=P))
                s01 = ep3.tile([P, Gj, P], dt.bfloat16, tag="s01",
                               padded_shape=[P, max(G), P])
                nc.sync.dma_start(s01[:, 0:Gj, :],
                                  s01T[:, coff[j] * P:(coff[j] + Gj) * P]
                                  .rearrange("p (g e) -> p g e", e=P))
                t01 = ep3.tile([P, Gj, P], dt.bfloat16, tag="t01",
                               padded_shape=[P, max(G), P])
                nc.sync.dma_start(t01[:, 0:Gj, :],
                                  t01T[:, coff[j] * P:(coff[j] + Gj) * P]
                                  .rearrange("p (g e) -> p g e", e=P))

                mg = eps.tile([P, Gj, HC + 8], dt.bfloat16, tag="mg",
                              padded_shape=[P, max(G), HC + 8])
                hT = eps.tile([P, Gj, HC], dt.bfloat16, tag="hT",
                              padded_shape=[P, max(G), HC])
                ert = stp.tile([P, Gj, 8], dt.float32, tag="ert",
                               padded_shape=[P, max(G), 8])
                for g in range(Gj):
                    psG = pp.tile([P, HC + 8], dt.float32, tag="psG", bufs=3)
                    # per-edge [h | asrc] = x[src] @ W1a[:, :264]
                    nc.tensor.matmul(psG[:], xe[:, g, :], w1a_sb[:, 0:HC + 8],
                                     start=True, stop=False)
                    # accumulate adst[dst(e)] onto the asrc columns
                    nc.tensor.matmul(psG[:, HC:HC + 8], t01[:, g, :],
                                     loc_adst[:, j, :], start=False, stop=True)
                    if g % 2 == 0:
                        nc.vector.tensor_copy(hT[:, g, :], psG[:, 0:HC])
                        nc.scalar.copy(ert[:, g, :], psG[:, HC:HC + 8])
                    else:
                        nc.scalar.copy(hT[:, g, :], psG[:, 0:HC])
                        nc.vector.tensor_copy(ert[:, g, :], psG[:, HC:HC + 8])
                # batched attention: lrelu -> exp -> weight messages
                er2t = stp.tile([P, Gj, 8], dt.float32, tag="er2t",
                                padded_shape=[P, max(G), 8])
                nc.vector.scalar_tensor_tensor(er2t[:], ert[:], NEG_SLOPE,
                                               ert[:], op0=OP.mult, op1=OP.max)
                alpt = stp.tile([P, Gj, 8], dt.float32, tag="alpt",
                                padded_shape=[P, max(G), 8])
                nc.scalar.activation(alpt[:], er2t[:], AF.Exp)
                nc.vector.tensor_tensor(
                    mg[:, :, 0:HC].rearrange("p g (h c) -> p g h c", h=HEADS),
                    hT[:, :, :].rearrange("p g (h c) -> p g h c", h=HEADS),
                    alpt[:].unsqueeze(3).broadcast_to([P, Gj, HEADS, HID]),
                    op=OP.mult)
                nc.vector.tensor_copy(mg[:, :, HC:HC + 8], alpt[:])
                ps_o = pp.tile([P, HC + 8], dt.float32, tag="ps_o", bufs=3)
                for g in range(Gj):
                    nc.tensor.matmul(ps_o[:], s01[:, g, :], mg[:, g, :],
                                     start=(g == 0), stop=(g == Gj - 1))

                # normalize + bias + ELU
                den = stp.tile([P, 8], dt.float32, tag="den")
                nc.vector.tensor_scalar(den[:], ps_o[:, HC:HC + 8], 1e-16, None,
                                        op0=OP.add)
                rec = stp.tile([P, 8], dt.float32, tag="rec")
                nc.vector.reciprocal(rec[:], den[:])
                o1 = stp.tile([P, HC], dt.float32, tag="o1")
                nc.vector.tensor_tensor(
                    o1[:].rearrange("p (h c) -> p h c", h=HEADS),
                    ps_o[:, 0:HC].rearrange("p (h c) -> p h c", h=HEADS),
                    rec[:].unsqueeze(2).broadcast_to([P, HEADS, HID]),
                    op=OP.mult)
                o1b = stp.tile([P, HC], dt.float32, tag="o1b")
                nc.vector.tensor_tensor(o1b[:], o1[:], b1_sb[:], op=OP.add)
                # ELU(x) = max(x,0) + exp(min(x,0)) - 1
                en = stp.tile([P, HC], dt.float32, tag="en")
                nc.vector.tensor_scalar(en[:], o1b[:], 0.0, None, op0=OP.min)
                ex = stp.tile([P, HC], dt.float32, tag="ex")
                nc.scalar.activation(ex[:], en[:], AF.Exp)
                h2a = stp.tile([P, HC], dt.float32, tag="h2a")
                nc.vector.scalar_tensor_tensor(h2a[:], o1b[:], 0.0, ex[:],
                                               op0=OP.max, op1=OP.add)
                h2b = stp.tile([P, HC], dt.bfloat16, tag="h2b")
                nc.vector.tensor_scalar(h2b[:], h2a[:], 1.0, None,
                                        op0=OP.subtract)

                # z = [h2 @ W2 | s2 | t2] (bf16)
                h2T = stp.tile([P, 2, P], dt.bfloat16, tag="h2T")
                for k in range(2):
                    pst = pp.tile([P, P], dt.bfloat16, tag="pst", bufs=1)
                    nc.tensor.transpose(pst[:], h2b[:, k * P:(k + 1) * P], ident[:])
                    nc.vector.tensor_copy(h2T[:, k, :], pst[:])
                ps2 = pp.tile([P, OUT_CH + 2], dt.float32, tag="ps2", bufs=1)
                for k in range(2):
                    nc.tensor.matmul(ps2[:], h2T[:, k, :], w2a_sb[:, k, :],
                                     start=(k == 0), stop=(k == 1))
                row2 = stp.tile([P, ROWZ], dt.bfloat16, tag="row2")
                nc.vector.memset(row2[:, OUT_CH + 2:ROWZ], 0.0)
                nc.vector.tensor_copy(row2[:, 0:OUT_CH + 2], ps2[:])
                nc.vector.tensor_copy(loc_adst2[:, j, :],
                                      ps2[:, OUT_CH + 1:OUT_CH + 2])
                nc.sync.dma_start(
                    h2rows[:, :].rearrange("(t p) r -> p t r", p=P)[:, j, :],
                    row2[:])

                # group AllGather as soon as its 5 tiles are done, then
                # repack rank-major group rows into node-ordered h2loc
                if (j + 1) % TPG == 0:
                    q = j // TPG
                    nc.gpsimd.collective_compute(
                        "AllGather", OP.bypass,
                        replica_groups=[list(range(N_CORES))],
                        ins=[h2rows[q * TPG * P:(q + 1) * TPG * P, :].opt()],
                        outs=[h2ag[q][:, :].opt()])
                    for b in range(-(-TILES_PER_CORE // TPG)):
                        ensure_block(q, b * TPG)
                cnt = 0
                for qj in list(pending):
                    if cnt == 4:
                        break
                    q2, j2 = qj
                    if j >= max((q2 + 1) * TPG, j2 + 1):
                        emit_l2(q2, j2)
                        pending.remove(qj)
                        cnt += 1

            for qj in list(pending):
                emit_l2(*qj)

    nc.compile()
    return nc


def kernel(x, edge_index, W1, a_src1, a_dst1, b1, W2, a_src2, a_dst2, b2):
    x = np.asarray(x, dtype=np.float32)
    W1 = np.asarray(W1, dtype=np.float32)
    a_src1 = np.asarray(a_src1, dtype=np.float32)
    a_dst1 = np.asarray(a_dst1, dtype=np.float32)
    b1 = np.asarray(b1, dtype=np.float32)
    W2 = np.asarray(W2, dtype=np.float32)
    a_src2 = np.asarray(a_src2, dtype=np.float32)
    a_dst2 = np.asarray(a_dst2, dtype=np.float32)
    b2 = np.asarray(b2, dtype=np.float32)

    G, G2, meta = _prep_edges(edge_index)

    A1 = np.zeros((HC, 16), np.float32)
    for h in range(HEADS):
        A1[h * HID:(h + 1) * HID, h] = a_src1[h]
        A1[h * HID:(h + 1) * HID, 8 + h] = a_dst1[h]
    W1a = np.concatenate([W1, W1 @ A1], axis=1).astype(bf16)
    W2a = np.concatenate([W2, W2 @ a_src2.T, W2 @ a_dst2.T], axis=1).astype(bf16)

    xT = np.zeros((P, N_PAD), np.float32)
    xT[:, :N_NODES] = x.T
    xTb = xT.astype(bf16)
    b1r = np.ascontiguousarray(np.tile(b1[None, :], (P, 1)).astype(np.float32))
    b2r = np.ascontiguousarray(np.tile(b2[None, :], (P, 1)).astype(np.float32))

    nc = _build_program(G, G2)

    in_maps = []
    for c in range(N_CORES):
        in_maps.append({
            "xeT": np.ascontiguousarray(xTb[:, meta[c]["src"]]),
            "xlT": np.ascontiguousarray(
                xTb[:, c * LOC_NODES:(c + 1) * LOC_NODES]),
            "W1a": np.ascontiguousarray(W1a),
            "W2a": np.ascontiguousarray(W2a),
            "b1r": b1r, "b2r": b2r,
            "s01": meta[c]["s01"], "t01": meta[c]["t01"],
            "idx2": meta[c]["idx2"], "s012": meta[c]["s012"],
            "t012": meta[c]["t012"],
        })

    trace = bool(os.environ.get("KERNEL_TRACE"))
    res = run_bass_kernel_spmd(nc, in_maps, core_ids=list(range(N_CORES)),
                               trace=trace,
                               tmpdir=os.environ.get("KERNEL_TRACE_DIR") or None)
    globals()["LAST_RESULTS"] = res
    out = np.concatenate([res.results[c]["out"] for c in range(N_CORES)], axis=0)
    return out[:N_NODES]


# revision 27
# speedup vs baseline: 1.2690x; 1.2690x over previous
"""2-layer GAT (PyG GATConv-style) on 8 Trainium2 NeuronCores.

Sharding: nodes padded to 160 tiles of 128; core c owns dst-tiles
[20c, 20c+20). Edges (incl. self-loops) live on the core owning their
destination, sorted by dst, padded to 128-edge chunks (G[j] chunks for
dst-tile slot j, identical across cores so all cores run one program).

Layer 1 needs h[src] = (x @ W1)[src] per edge; instead of gathering on
device, the host uploads x.T[:, src(e)] per edge (xeT) and the kernel
computes per-edge hidden rows with a per-chunk GEMM in bf16 — no
dma_gather in layer 1 (gpsimd dma_gather costs ~8ns/idx, measured).
Segment-softmax and scatter-aggregate run per dst-tile as one-hot bf16
matmuls (s01 scatter / t01 expand, host-built, streamed as bf16).

Between layers, per-node rows z = [h2@W2 | asrc2 | adst2] (bf16, padded
to 256B) are AllGathered in 4 groups of 5 dst-tiles so the collective
overlaps layer-1 compute. Layer 2 fetches z[src(e)] per edge with
dma_gather (unavoidable; index-rate-bound) and aggregates with the same
one-hot matmuls.
"""

import os

import numpy as np
import ml_dtypes

import concourse.bass as bass
import concourse.mybir as mybir
import concourse.tile as tile
from concourse import bacc
from concourse.bass_utils import run_bass_kernel_spmd

dt = mybir.dt
bf16 = ml_dtypes.bfloat16

N_CORES = 8
N_NODES = 20000
IN_CH = 128
HID = 32
HEADS = 8
HC = HEADS * HID  # 256
OUT_CH = 64
NEG_SLOPE = 0.2

P = 128
TILES_PER_CORE = 20
N_PAD = N_CORES * TILES_PER_CORE * P  # 20480
LOC_NODES = TILES_PER_CORE * P  # 2560
NGRP = 4                  # AllGather groups
TPG = TILES_PER_CORE // NGRP  # tiles per group
ROWZ = 128                # z row: [h2W2(64) | s2 | t2 | pad] bf16 -> 256B

AF = mybir.ActivationFunctionType
OP = mybir.AluOpType


def _prep_edges(edge_index):
    src = np.asarray(edge_index[0], dtype=np.int64)
    dst = np.asarray(edge_index[1], dtype=np.int64)
    loops = np.arange(N_NODES, dtype=np.int64)
    src = np.concatenate([src, loops])
    dst = np.concatenate([dst, loops])

    order = np.lexsort((src, dst))
    src, dst = src[order], dst[order]
    tile_of = dst // P
    core_of = np.minimum(tile_of // TILES_PER_CORE, N_CORES - 1)

    per = [[None] * TILES_PER_CORE for _ in range(N_CORES)]
    for c in range(N_CORES):
        mc = core_of == c
        sc, tc_, dc = src[mc], tile_of[mc], dst[mc]
        for j in range(TILES_PER_CORE):
            gt = c * TILES_PER_CORE + j
            mt = tc_ == gt
            per[c][j] = (sc[mt], dc[mt] - gt * P)

    G = [max(max(1, -(-len(per[c][j][0]) // P)) for c in range(N_CORES))
         for j in range(TILES_PER_CORE)]

    # layer-2: edges of (core, dst-tile) split by src AllGather group
    per2 = [[[None] * NGRP for _ in range(TILES_PER_CORE)]
            for _ in range(N_CORES)]
    for c in range(N_CORES):
        for j in range(TILES_PER_CORE):
            s, dl = per[c][j]
            qof = (s % LOC_NODES) // (TPG * P)
            for q in range(NGRP):
                m = qof == q
                per2[c][j][q] = (s[m], dl[m])
    G2 = [[max(max(1, -(-len(per2[c][j][q][0]) // P)) for c in range(N_CORES))
           for q in range(NGRP)] for j in range(TILES_PER_CORE)]

    def onehots(dlp):
        s01 = (dlp[:, None] == np.arange(P)[None, :])
        return s01.astype(bf16), np.ascontiguousarray(s01.T).astype(bf16)

    meta = []
    for c in range(N_CORES):
        srcs, s01s, t01s = [], [], []
        for j in range(TILES_PER_CORE):
            s, dl = per[c][j]
            n_pad = G[j] * P
            sp = np.zeros(n_pad, dtype=np.int64)
            sp[: len(s)] = s
            srcs.append(sp)
            dlp = np.full(n_pad, 200, dtype=np.int64)
            dlp[: len(dl)] = dl
            for g in range(G[j]):
                a, b = onehots(dlp[g * P:(g + 1) * P])
                s01s.append(a)
                t01s.append(b)
        idx2_cols, s01s2, t01s2 = [], [], []
        for q in range(NGRP):
            for j in range(TILES_PER_CORE):
                s, dl = per2[c][j][q]
                n_pad = G2[j][q] * P
                sp = np.zeros(n_pad, dtype=np.int64)
                sp[: len(s)] = s
                dlp = np.full(n_pad, 200, dtype=np.int64)
                dlp[: len(dl)] = dl
                # row within h2ag[q]: rank-major
                rows = (sp // LOC_NODES) * (TPG * P) + (sp % LOC_NODES) - q * TPG * P
                rows[len(s):] = 0
                idx16 = rows.astype(np.int16).reshape(n_pad // 16, 16).T
                idx2_cols.append(np.tile(idx16, (8, 1)))
                for g in range(G2[j][q]):
                    a, b = onehots(dlp[g * P:(g + 1) * P])
                    s01s2.append(a)
                    t01s2.append(b)
        meta.append({
            "src": np.concatenate(srcs),
            "s01": np.ascontiguousarray(np.stack(s01s, 1).reshape(P, -1)),
            "t01": np.ascontiguousarray(np.stack(t01s, 1).reshape(P, -1)),
            "idx2": np.ascontiguousarray(np.concatenate(idx2_cols, axis=1)),
            "s012": np.ascontiguousarray(np.stack(s01s2, 1).reshape(P, -1)),
            "t012": np.ascontiguousarray(np.stack(t01s2, 1).reshape(P, -1)),
        })
    return G, G2, meta


def _build_program(G, G2):
    NCH = sum(G)
    NCH2 = sum(G2[j][q] for j in range(TILES_PER_CORE) for q in range(NGRP))
    nc = bacc.Bacc(None, target_bir_lowering=False, debug=False)

    xeT = nc.dram_tensor("xeT", [P, NCH * P], dt.bfloat16, kind="ExternalInput")
    xlT = nc.dram_tensor("xlT", [P, LOC_NODES], dt.bfloat16, kind="ExternalInput")
    W1a = nc.dram_tensor("W1a", [P, HC + 16], dt.bfloat16, kind="ExternalInput")
    W2a = nc.dram_tensor("W2a", [HC, OUT_CH + 2], dt.bfloat16, kind="ExternalInput")
    b1r = nc.dram_tensor("b1r", [P, HC], dt.float32, kind="ExternalInput")
    b2r = nc.dram_tensor("b2r", [P, OUT_CH], dt.float32, kind="ExternalInput")
    s01T = nc.dram_tensor("s01", [P, NCH * P], dt.bfloat16, kind="ExternalInput")
    t01T = nc.dram_tensor("t01", [P, NCH * P], dt.bfloat16, kind="ExternalInput")
    idx2T = nc.dram_tensor("idx2", [P, 8 * NCH2], dt.int16, kind="ExternalInput")
    s012T = nc.dram_tensor("s012", [P, NCH2 * P], dt.bfloat16, kind="ExternalInput")
    t012T = nc.dram_tensor("t012", [P, NCH2 * P], dt.bfloat16, kind="ExternalInput")
    outT = nc.dram_tensor("out", [LOC_NODES, OUT_CH], dt.float32, kind="ExternalOutput")

    coff = [0]
    for j in range(1, TILES_PER_CORE + 1):
        coff.append(coff[-1] + G[j - 1])
    coff2 = {}
    off = 0
    for q in range(NGRP):
        for j in range(TILES_PER_CORE):
            coff2[(q, j)] = off
            off += G2[j][q]
    maxG2 = max(G2[j][q] for j in range(TILES_PER_CORE) for q in range(NGRP))

    with tile.TileContext(nc) as tc:
        with (
            tc.tile_pool(name="const", bufs=1) as cp,
            tc.tile_pool(name="stage", bufs=4) as stp,
            tc.tile_pool(name="edges", bufs=3) as eps,
            tc.tile_pool(name="edges3", bufs=5) as ep3,
            tc.tile_pool(name="ps", bufs=1, space="PSUM") as pp,
            tc.tile_pool(name="dram", bufs=1, space="DRAM") as dp,
        ):
            # ---- constants ----
            w1a_sb = cp.tile([P, HC + 16], dt.bfloat16)
            nc.sync.dma_start(w1a_sb[:], W1a[:, :])
            w2a_sb = cp.tile([P, 2, OUT_CH + 2], dt.bfloat16)
            nc.sync.dma_start(w2a_sb[:],
                              W2a[:, :].rearrange("(k p) n -> p k n", p=P))
            b1_sb = cp.tile([P, HC], dt.float32)
            nc.sync.dma_start(b1_sb[:], b1r[:, :])
            b2_sb = cp.tile([P, OUT_CH], dt.float32)
            nc.sync.dma_start(b2_sb[:], b2r[:, :])
            idx2_sb = cp.tile([P, 8 * NCH2], dt.int16)
            nc.sync.dma_start(idx2_sb[:], idx2T[:, :])
            xl_sb = cp.tile([P, LOC_NODES], dt.bfloat16)
            nc.sync.dma_start(xl_sb[:], xlT[:, :])

            iot_row = cp.tile([P, P], dt.float32)
            nc.gpsimd.iota(iot_row[:], pattern=[[1, P]], base=0,
                           channel_multiplier=0,
                           allow_small_or_imprecise_dtypes=True)
            iot_col = cp.tile([P, 1], dt.float32)
            nc.gpsimd.iota(iot_col[:], pattern=[[0, 1]], base=0,
                           channel_multiplier=1,
                           allow_small_or_imprecise_dtypes=True)
            ident = cp.tile([P, P], dt.bfloat16)
            nc.vector.tensor_scalar(ident[:], iot_row[:], iot_col[:], None,
                                    op0=OP.is_equal)

            h2rows = dp.tile([LOC_NODES, ROWZ], dt.bfloat16)
            h2ag = [dp.tile([N_CORES * TPG * P, ROWZ], dt.bfloat16,
                            addr_space="Shared", name=f"h2ag{q}")
                    for q in range(NGRP)]

            # ---- local adst1 per dst-tile (from local x) ----
            loc_adst = cp.tile([P, TILES_PER_CORE, 8], dt.bfloat16)
            for j in range(TILES_PER_CORE):
                psL = pp.tile([P, 8], dt.float32, tag="ps2", bufs=1,
                              padded_shape=[P, OUT_CH + 2])
                nc.tensor.matmul(psL[:], xl_sb[:, j * P:(j + 1) * P],
                                 w1a_sb[:, HC + 8:HC + 16], start=True, stop=True)
                nc.vector.tensor_copy(loc_adst[:, j, :], psL[:])
            loc_adst2 = cp.tile([P, TILES_PER_CORE, 1], dt.bfloat16)

            # ---- layer 2 sub-tile emission (interleaved with layer 1) ----
            acc2 = cp.tile([P, TILES_PER_CORE, OUT_CH + 1], dt.float32)

            def emit_l2(q, j):
                Gq = G2[j][q]
                ni = Gq * P
                off = coff2[(q, j)]
                hg2 = ep3.tile([P, Gq, ROWZ], dt.bfloat16, tag="hg2",
                               padded_shape=[P, maxG2, ROWZ])
                nc.gpsimd.dma_gather(hg2[:, 0:Gq, :], h2ag[q][:, :],
                                     idx2_sb[:, 8 * off:8 * (off + Gq)],
                                     num_idxs=ni, num_idxs_reg=ni,
                                     elem_size=ROWZ, single_packet=False)
                s01 = ep3.tile([P, Gq, P], dt.bfloat16, tag="s01b",
                               padded_shape=[P, maxG2, P])
                nc.sync.dma_start(s01[:, 0:Gq, :],
                                  s012T[:, off * P:(off + Gq) * P]
                                  .rearrange("p (g e) -> p g e", e=P))
                t01 = ep3.tile([P, Gq, P], dt.bfloat16, tag="t01b",
                               padded_shape=[P, maxG2, P])
                nc.sync.dma_start(t01[:, 0:Gq, :],
                                  t012T[:, off * P:(off + Gq) * P]
                                  .rearrange("p (g e) -> p g e", e=P))

                mg2 = eps.tile([P, Gq, OUT_CH + 1], dt.bfloat16, tag="mg2",
                               padded_shape=[P, maxG2, OUT_CH + 1])
                a2t = stp.tile([P, Gq, 1], dt.float32, tag="a2t",
                               padded_shape=[P, maxG2, 1])
                for g in range(Gq):
                    ps_a2 = pp.tile([P, 1], dt.float32, tag="psG", bufs=3,
                                    padded_shape=[P, HC + 8])
                    nc.tensor.matmul(ps_a2[:], t01[:, g, :],
                                     loc_adst2[:, j, :], start=True, stop=True)
                    nc.vector.tensor_tensor(a2t[:, g, :],
                                            hg2[:, g, OUT_CH:OUT_CH + 1],
                                            ps_a2[:], op=OP.add)
                e2b = stp.tile([P, Gq], dt.float32, tag="e2b",
                               padded_shape=[P, maxG2])
                nc.vector.scalar_tensor_tensor(e2b[:], a2t[:, :, 0], NEG_SLOPE,
                                               a2t[:, :, 0],
                                               op0=OP.mult, op1=OP.max)
                al2t = stp.tile([P, Gq], dt.float32, tag="al2t",
                                padded_shape=[P, maxG2])
                nc.scalar.activation(al2t[:], e2b[:], AF.Exp)
                nc.vector.tensor_tensor(
                    mg2[:, :, 0:OUT_CH], hg2[:, 0:Gq, 0:OUT_CH],
                    al2t[:].unsqueeze(2).broadcast_to([P, Gq, OUT_CH]),
                    op=OP.mult)
                nc.vector.tensor_copy(mg2[:, :, OUT_CH:OUT_CH + 1],
                                      al2t[:].unsqueeze(2))
                ps3 = pp.tile([P, OUT_CH + 1], dt.float32, tag="ps_o", bufs=3,
                              padded_shape=[P, HC + 8])
                for g in range(Gq):
                    nc.tensor.matmul(ps3[:], s01[:, g, :], mg2[:, g, :],
                                     start=(g == 0), stop=(g == Gq - 1))
                if q == 0:
                    nc.vector.tensor_copy(acc2[:, j, :], ps3[:])
                else:
                    nc.vector.tensor_tensor(acc2[:, j, :], acc2[:, j, :],
                                            ps3[:], op=OP.add)
                if q == NGRP - 1:
                    den2 = stp.tile([P, 1], dt.float32, tag="den2")
                    nc.vector.tensor_scalar(den2[:],
                                            acc2[:, j, OUT_CH:OUT_CH + 1],
                                            1e-16, None, op0=OP.add)
                    rec2 = stp.tile([P, 1], dt.float32, tag="rec2")
                    nc.vector.reciprocal(rec2[:], den2[:])
                    o2 = stp.tile([P, OUT_CH], dt.float32, tag="o2")
                    nc.vector.tensor_tensor(o2[:], acc2[:, j, 0:OUT_CH],
                                            rec2[:].broadcast_to([P, OUT_CH]),
                                            op=OP.mult)
                    o2b = stp.tile([P, OUT_CH], dt.float32, tag="o2b")
                    nc.vector.tensor_tensor(o2b[:], o2[:], b2_sb[:], op=OP.add)
                    nc.sync.dma_start(
                        outT[:, :].rearrange("(t p) r -> p t r", p=P)[:, j, :],
                        o2b[:])

            pending = [(q2, j2) for q2 in range(NGRP)
                       for j2 in range(TILES_PER_CORE)]

            # ---- layer 1 ----
            for j in range(TILES_PER_CORE):
                Gj = G[j]
                xe = ep3.tile([P, Gj, P], dt.bfloat16, tag="xe",
                              padded_shape=[P, max(G), P])
                nc.sync.dma_start(xe[:, 0:Gj, :],
                                  xeT[:, coff[j] * P:(coff[j] + Gj) * P]
                                  .rearrange("p (g e) -> p g e", e=P))
                s01 = ep3.tile([P, Gj, P], dt.bfloat16, tag="s01",
                               padded_shape=[P, max(G), P])
                nc.sync.dma_start(s01[:, 0:Gj, :],
                                  s01T[:, coff[j] * P:(coff[j] + Gj) * P]
                                  .rearrange("p (g e) -> p g e", e=P))
                t01 = ep3.tile([P, Gj, P], dt.bfloat16, tag="t01",
                               padded_shape=[P, max(G), P])
                nc.sync.dma_start(t01[:, 0:Gj, :],
                                  t01T[:, coff[j] * P:(coff[j] + Gj) * P]
                                  .rearrange("p (g e) -> p g e", e=P))

                mg = eps.tile([P, Gj, HC + 8], dt.bfloat16, tag="mg",
                              padded_shape=[P, max(G), HC + 8])
                hT = eps.tile([P, Gj, HC], dt.bfloat16, tag="hT",
                              padded_shape=[P, max(G), HC])
                ert = stp.tile([P, Gj, 8], dt.float32, tag="ert",
                               padded_shape=[P, max(G), 8])
                for g in range(Gj):
                    psG = pp.tile([P, HC + 8], dt.float32, tag="psG", bufs=3)
                    # per-edge [h | asrc] = x[src] @ W1a[:, :264]
                    nc.tensor.matmul(psG[:], xe[:, g, :], w1a_sb[:, 0:HC + 8],
                                     start=True, stop=False)
                    # accumulate adst[dst(e)] onto the asrc columns
                    nc.tensor.matmul(psG[:, HC:HC + 8], t01[:, g, :],
                                     loc_adst[:, j, :], start=False, stop=True)
                    if g % 2 == 0:
                        nc.vector.tensor_copy(hT[:, g, :], psG[:, 0:HC])
                        nc.scalar.copy(ert[:, g, :], psG[:, HC:HC + 8])
                    else:
                        nc.scalar.copy(hT[:, g, :], psG[:, 0:HC])
                        nc.vector.tensor_copy(ert[:, g, :], psG[:, HC:HC + 8])
                # batched attention: lrelu -> exp -> weight messages
                er2t = stp.tile([P, Gj, 8], dt.float32, tag="er2t",
                                padded_shape=[P, max(G), 8])
                nc.vector.scalar_tensor_tensor(er2t[:], ert[:], NEG_SLOPE,
                                               ert[:], op0=OP.mult, op1=OP.max)
                alpt = stp.tile([P, Gj, 8], dt.float32, tag="alpt",
                                padded_shape=[P, max(G), 8])
                nc.scalar.activation(alpt[:], er2t[:], AF.Exp)
                nc.vector.tensor_tensor(
                    mg[:, :, 0:HC].rearrange("p g (h c) -> p g h c", h=HEADS),
                    hT[:, :, :].rearrange("p g (h c) -> p g h c", h=HEADS),
                    alpt[:].unsqueeze(3).broadcast_to([P, Gj, HEADS, HID]),
                    op=OP.mult)
                nc.vector.tensor_copy(mg[:, :, HC:HC + 8], alpt[:])
                ps_o = pp.tile([P, HC + 8], dt.float32, tag="ps_o", bufs=3)
                for g in range(Gj):
                    nc.tensor.matmul(ps_o[:], s01[:, g, :], mg[:, g, :],
                                     start=(g == 0), stop=(g == Gj - 1))

                # normalize + bias + ELU
                den = stp.tile([P, 8], dt.float32, tag="den")
                nc.vector.tensor_scalar(den[:], ps_o[:, HC:HC + 8], 1e-16, None,
                                        op0=OP.add)
                rec = stp.tile([P, 8], dt.float32, tag="rec")
                nc.vector.reciprocal(rec[:], den[:])
                o1 = stp.tile([P, HC], dt.float32, tag="o1")
                nc.vector.tensor_tensor(
                    o1[:].rearrange("p (h c) -> p h c", h=HEADS),
                    ps_o[:, 0:HC].rearrange("p (h c) -> p h c", h=HEADS),
                    rec[:].unsqueeze(2).broadcast_to([P, HEADS, HID]),
                    op=OP.mult)
                o1b = stp.tile([P, HC], dt.float32, tag="o1b")
                nc.vector.tensor_tensor(o1b[:], o1[:], b1_sb[:], op=OP.add)
                # ELU(x) = max(x,0) + exp(min(x,0)) - 1
                en = stp.tile([P, HC], dt.float32, tag="en")
                nc.vector.tensor_scalar(en[:], o1b[:], 0.0, None, op0=OP.min)
                ex = stp.tile([P, HC], dt.float32, tag="ex")
                nc.scalar.activation(ex[:], en[:], AF.Exp)
                h2a = stp.tile([P, HC], dt.float32, tag="h2a")
                nc.vector.scalar_tensor_tensor(h2a[:], o1b[:], 0.0, ex[:],
                                               op0=OP.max, op1=OP.add)
                h2b = stp.tile([P, HC], dt.bfloat16, tag="h2b")
                nc.vector.tensor_scalar(h2b[:], h2a[:], 1.0, None,
                                        op0=OP.subtract)

                # z = [h2 @ W2 | s2 | t2] (bf16)
                h2T = stp.tile([P, 2, P], dt.bfloat16, tag="h2T")
                for k in range(2):
                    pst = pp.tile([P, P], dt.bfloat16, tag="pst", bufs=1)
                    nc.tensor.transpose(pst[:], h2b[:, k * P:(k + 1) * P], ident[:])
                    nc.vector.tensor_copy(h2T[:, k, :], pst[:])
                ps2 = pp.tile([P, OUT_CH + 2], dt.float32, tag="ps2", bufs=1)
                for k in range(2):
                    nc.tensor.matmul(ps2[:], h2T[:, k, :], w2a_sb[:, k, :],
                                     start=(k == 0), stop=(k == 1))
                row2 = stp.tile([P, ROWZ], dt.bfloat16, tag="row2")
                nc.vector.memset(row2[:, OUT_CH + 2:ROWZ], 0.0)
                nc.vector.tensor_copy(row2[:, 0:OUT_CH + 2], ps2[:])
                nc.vector.tensor_copy(loc_adst2[:, j, :],
                                      ps2[:, OUT_CH + 1:OUT_CH + 2])
                nc.sync.dma_start(
                    h2rows[:, :].rearrange("(t p) r -> p t r", p=P)[:, j, :],
                    row2[:])

                # group AllGather as soon as its 5 tiles are done, then
                # repack rank-major group rows into node-ordered h2loc
                if (j + 1) % TPG == 0:
                    q = j // TPG
                    nc.gpsimd.collective_compute(
                        "AllGather", OP.bypass,
                        replica_groups=[list(range(N_CORES))],
                        ins=[h2rows[q * TPG * P:(q + 1) * TPG * P, :].opt()],
                        outs=[h2ag[q][:, :].opt()])
                cnt = 0
                for qj in list(pending):
                    if cnt == 4:
                        break
                    q2, j2 = qj
                    if j >= max((q2 + 1) * TPG, j2 + 1):
                        emit_l2(q2, j2)
                        pending.remove(qj)
                        cnt += 1

            for qj in list(pending):
                emit_l2(*qj)

    nc.compile()
    return nc


def kernel(x, edge_index, W1, a_src1, a_dst1, b1, W2, a_src2, a_dst2, b2):
    x = np.asarray(x, dtype=np.float32)
    W1 = np.asarray(W1, dtype=np.float32)
    a_src1 = np.asarray(a_src1, dtype=np.float32)
    a_dst1 = np.asarray(a_dst1, dtype=np.float32)
    b1 = np.asarray(b1, dtype=np.float32)
    W2 = np.asarray(W2, dtype=np.float32)
    a_src2 = np.asarray(a_src2, dtype=np.float32)
    a_dst2 = np.asarray(a_dst2, dtype=np.float32)
    b2 = np.asarray(b2, dtype=np.float32)

    G, G2, meta = _prep_edges(edge_index)

    A1 = np.zeros((HC, 16), np.float32)
    for h in range(HEADS):
        A1[h * HID:(h + 1) * HID, h] = a_src1[h]
        A1[h * HID:(h + 1) * HID, 8 + h] = a_dst1[h]
    W1a = np.concatenate([W1, W1 @ A1], axis=1).astype(bf16)
    W2a = np.concatenate([W2, W2 @ a_src2.T, W2 @ a_dst2.T], axis=1).astype(bf16)

    xT = np.zeros((P, N_PAD), np.float32)
    xT[:, :N_NODES] = x.T
    xTb = xT.astype(bf16)
    b1r = np.ascontiguousarray(np.tile(b1[None, :], (P, 1)).astype(np.float32))
    b2r = np.ascontiguousarray(np.tile(b2[None, :], (P, 1)).astype(np.float32))

    nc = _build_program(G, G2)

    in_maps = []
    for c in range(N_CORES):
        in_maps.append({
            "xeT": np.ascontiguousarray(xTb[:, meta[c]["src"]]),
            "xlT": np.ascontiguousarray(
                xTb[:, c * LOC_NODES:(c + 1) * LOC_NODES]),
            "W1a": np.ascontiguousarray(W1a),
            "W2a": np.ascontiguousarray(W2a),
            "b1r": b1r, "b2r": b2r,
            "s01": meta[c]["s01"], "t01": meta[c]["t01"],
            "idx2": meta[c]["idx2"], "s012": meta[c]["s012"],
            "t012": meta[c]["t012"],
        })

    trace = bool(os.environ.get("KERNEL_TRACE"))
    res = run_bass_kernel_spmd(nc, in_maps, core_ids=list(range(N_CORES)),
                               trace=trace,
                               tmpdir=os.environ.get("KERNEL_TRACE_DIR") or None)
    globals()["LAST_RESULTS"] = res
    out = np.concatenate([res.results[c]["out"] for c in range(N_CORES)], axis=0)
    return out[:N_NODES]
